# revision 2
# baseline (speedup 1.0000x reference)
"""Trainium2 Bass kernel for nn_ComposedFeatureTransformer (NNUE-style double
feature transformer: sparse gather-accumulate + bias, perspective concat, clip,
psqt head).

Strategy: data-parallel over batch across 8 NeuronCores (512 items/core, table
replicated). Per core, items are processed in 4 tiles of 128 (one item per SBUF
partition); each tile needs two 32-row weighted sums (w/b perspectives).

The table is cast to fp16 on host: the kernel is memory-bound on gather
traffic, and halving bytes-per-row halves DMA time; fp16 accumulation keeps
absmax error ~2.4e-3 of output scale (tolerance 2e-2). Rows are fetched with
indirect DMA (one [128, D] fp16 row-gather per feature slot) into a ring of
staging tiles; DVE accumulates staging tiles into the per-perspective
accumulator (first add seeds with the broadcast bias; fp16 operands get the
DVE 2x packed mode), then combines perspectives (us/them mix + clip + psqt)
into an fp16 output tile that SP (HWDGE) streams to DRAM; host upcasts to f32.
All DMA writes are plain copies — no in-DMA accumulate (the SDMA CCE
read-modify-write is unsound for multi-packet rows). Per-core gather traffic:
512*64 rows * 6160 B ~= 202 MB.

Requires w_values/b_values == 1 (guaranteed by the problem spec fill); falls
back to a host computation otherwise so kernel() stays correct on any input.
"""
import sys

if '/opt/trn_rl_repo' not in sys.path:
    sys.path.insert(0, '/opt/trn_rl_repo')

import numpy as np

import concourse.bass as bass
import concourse.mybir as mybir
from concourse.bass_utils import run_bass_kernel_spmd

L1 = 3072
NPSQT = 8
D = L1 + NPSQT            # 3080
V = 45056                 # table rows
K = 32                    # active features per perspective
B = 4096                  # batch
NCORES = 8
BPC = B // NCORES         # 512 items per core
NT = BPC // 128           # 4 item-tiles per core
OUTD = 2 * L1 + NPSQT     # 6152
# PAIR=2 (two rows per indirect DMA) was measured cost-model-neutral —
# descriptor generation is fully hidden behind the transfer stream — so the
# hardware-validated single-row configuration is kept.
PAIR = 1                  # table rows fetched per indirect DMA
S = 6                     # staging ring depth (slots of PAIR rows)

f32 = mybir.dt.float32
f16 = mybir.dt.float16
i32 = mybir.dt.int32

_CACHE = {}


def build_nc(ft_max: float, repeat: int = 1):
    """Build the single-core Bass program (shared SPMD across all 8 cores).

    repeat>1 re-runs the whole compute that many times (for timing the
    on-device portion by slope; the output is simply rewritten).
    """
    nc = bass.Bass()
    table = nc.declare_dram_parameter("table", [V, D], f16, isOutput=False)
    biasb = nc.declare_dram_parameter("biasb", [128, D], f16, isOutput=False)
    idxs = nc.declare_dram_parameter("idxs", [128, 2 * NT * K], i32, isOutput=False)
    usth = nc.declare_dram_parameter("usth", [128, 3 * NT], f32, isOutput=False)
    out = nc.declare_dram_parameter("out", [BPC, OUTD], f16, isOutput=True)

    NG = repeat * 2 * NT * K      # total gathers
    LOADS = 48                    # idx + usth + bias preload sem ticks

    with (
        nc.sbuf_tensor([128, 2 * NT * K], i32) as idx_s,
        nc.sbuf_tensor([128, 3 * NT], f32) as usth_s,
        nc.sbuf_tensor([128, D], f16) as bias_s,
        nc.sbuf_tensor([128, D], f16) as acc_w,
        nc.sbuf_tensor([128, D], f16) as acc_b,
        nc.sbuf_tensor([128, S * PAIR * D], f16) as stage,
        nc.sbuf_tensor([128, OUTD], f16) as out_t0,
        nc.sbuf_tensor([128, OUTD], f16) as out_t1,
        nc.sbuf_tensor([128, L1], f16) as tmp,
        nc.semaphore("load_sem") as load_sem,
        nc.semaphore("gsem0") as gsem0,
        nc.semaphore("gsem1") as gsem1,
        nc.semaphore("gsem2") as gsem2,
        nc.semaphore("gsem3") as gsem3,
        nc.semaphore("gsem4") as gsem4,
        nc.semaphore("gsem5") as gsem5,
        nc.semaphore("dve_sem") as dve_sem,
        nc.semaphore("combine_sem") as combine_sem,
        nc.semaphore("osem0") as osem0,
        nc.semaphore("osem1") as osem1,
        nc.Block() as block,
    ):
        # one completion sem per staging slot / output-tile parity: a waiter's
        # threshold can only be satisfied by that slot's own DMA chain, so a
        # fast SDMA engine running ahead on *other* DMAs can't mask a slow
        # engine's unfinished descriptors (cumulative counts on one shared sem
        # are unsound across concurrently in-flight DMAs)
        gsem = [gsem0, gsem1, gsem2, gsem3, gsem4, gsem5][:S]
        assert len(gsem) == S
        osem = [osem0, osem1]
        out_t = [out_t0, out_t1]

        def stg(s, j=0):
            # row j of staging slot s
            return stage[:, (s * PAIR + j) * D:(s * PAIR + j + 1) * D]

        @block.gpsimd
        def _(g):
            g.dma_start(out=idx_s[:], in_=idxs[:]).then_inc(load_sem, 16)
            g.dma_start(out=usth_s[:], in_=usth[:]).then_inc(load_sem, 16)
            g.dma_start(out=bias_s[:], in_=biasb[:]).then_inc(load_sem, 16)
            # descriptor generation reads idx_s from SBUF: loads must land first
            g.wait_ge(load_sem, LOADS)
            assert K % PAIR == 0  # a DMA's rows never straddle units
            for gp in range(NG // PAIR):
                u, k0 = (gp * PAIR) // K, (gp * PAIR) % K
                t, p = (u // 2) % NT, u % 2
                col = (2 * t + p) * K + k0
                if gp >= S:
                    # DVE must have consumed the staging slot's previous tenant
                    g.wait_ge(dve_sem, (gp - S + 1) * PAIR)
                g.indirect_dma_start(
                    out=stage[:, (gp % S) * PAIR * D:((gp % S) + 1) * PAIR * D],
                    out_offset=None,
                    in_=table[:],
                    in_offset=bass.IndirectOffsetOnAxis(
                        ap=idx_s[:, col:col + PAIR], axis=0
                    ),
                ).then_inc(gsem[gp % S], 16)

        @block.vector
        def _(v):
            v.wait_ge(load_sem, LOADS)   # usth_s/bias_s resident
            for it in range(repeat * NT):
                t, pb = it % NT, it % 2
                for p, acc in ((0, acc_w), (1, acc_b)):
                    u = 2 * it + p
                    for k in range(K):
                        gi = u * K + k
                        gp, j = gi // PAIR, gi % PAIR
                        if j == 0:
                            v.wait_ge(gsem[gp % S], 16 * (gp // S + 1))
                        src = bias_s if k == 0 else acc
                        v.tensor_tensor(
                            out=acc[:], in0=src[:], in1=stg(gp % S, j),
                            op=mybir.AluOpType.add,
                        ).then_inc(dve_sem, 1)
                if it >= 2:
                    # SP must have drained out_t[pb] (tile it-2)
                    v.wait_ge(osem[pb], 16 * ((it - 2) // 2 + 1))
                w, b, o = acc_w, acc_b, out_t[pb]
                us = usth_s[:, t:t + 1]
                them = usth_s[:, NT + t:NT + t + 1]
                ush = usth_s[:, 2 * NT + t:2 * NT + t + 1]
                # o[:, :L1] = clip(us*w + them*b); o[:, L1:2L1] = clip(us*b + them*w)
                v.tensor_scalar_mul(tmp[:], b[:, :L1], them)
                v.scalar_tensor_tensor(
                    o[:, 0:L1], w[:, :L1], us, tmp[:],
                    op0=mybir.AluOpType.mult, op1=mybir.AluOpType.add,
                )
                v.tensor_scalar(
                    o[:, 0:L1], o[:, 0:L1], 0.0, ft_max,
                    op0=mybir.AluOpType.max, op1=mybir.AluOpType.min,
                )
                v.tensor_scalar_mul(tmp[:], w[:, :L1], them)
                v.scalar_tensor_tensor(
                    o[:, L1:2 * L1], b[:, :L1], us, tmp[:],
                    op0=mybir.AluOpType.mult, op1=mybir.AluOpType.add,
                )
                v.tensor_scalar(
                    o[:, L1:2 * L1], o[:, L1:2 * L1], 0.0, ft_max,
                    op0=mybir.AluOpType.max, op1=mybir.AluOpType.min,
                )
                # psqt = (w_psqt - b_psqt) * (us - 0.5); bias cancels in the diff
                v.tensor_tensor(
                    out=tmp[:, :NPSQT], in0=w[:, L1:D], in1=b[:, L1:D],
                    op=mybir.AluOpType.subtract,
                )
                v.tensor_scalar_mul(
                    o[:, 2 * L1:OUTD], tmp[:, :NPSQT], ush
                ).then_inc(combine_sem, 1)

        @block.sync
        def _(s):
            for it in range(repeat * NT):
                t, pb = it % NT, it % 2
                s.wait_ge(combine_sem, it + 1)
                s.dma_start(
                    out=out[t * 128:(t + 1) * 128, :], in_=out_t[pb][:]
                ).then_inc(osem[pb], 16)
            n = repeat * NT
            s.wait_ge(osem[0], 16 * ((n + 1) // 2))
            s.wait_ge(osem[1], 16 * (n // 2))

    return nc


def _prep_core_inputs(c, table, biasb, w_idx, b_idx, us, them):
    sl = slice(c * BPC, (c + 1) * BPC)
    wi = w_idx[sl].reshape(NT, 128, K)
    bi = b_idx[sl].reshape(NT, 128, K)
    blocks = []
    for t in range(NT):
        blocks.append(wi[t])
        blocks.append(bi[t])
    idxs = np.ascontiguousarray(np.concatenate(blocks, axis=1), dtype=np.int32)
    us_c = np.ascontiguousarray(us[sl, 0].reshape(NT, 128).T, dtype=np.float32)
    th_c = np.ascontiguousarray(them[sl, 0].reshape(NT, 128).T, dtype=np.float32)
    usth = np.concatenate([us_c, th_c, us_c - 0.5], axis=1).astype(np.float32)
    return {"table": table, "biasb": biasb, "idxs": idxs, "usth": usth}


def run_on_hw(w_indices, w_values, b_indices, b_values, us, them, ft_max_val,
              merged_weight, bias, trace=False, repeat=1):
    """Run the device kernel; returns (output [B, OUTD], BassKernelResults)."""
    ft_max = float(np.asarray(ft_max_val))
    key = ("nc", ft_max, repeat)
    if key not in _CACHE:
        _CACHE[key] = build_nc(ft_max, repeat)
    nc = _CACHE[key]

    table = np.ascontiguousarray(merged_weight, dtype=np.float16)
    biasb = np.ascontiguousarray(
        np.broadcast_to(np.asarray(bias, dtype=np.float16), (128, D))
    )
    w_idx = np.asarray(w_indices, dtype=np.int64)
    b_idx = np.asarray(b_indices, dtype=np.int64)
    us = np.asarray(us, dtype=np.float32)
    them = np.asarray(them, dtype=np.float32)

    in_maps = [
        _prep_core_inputs(c, table, biasb, w_idx, b_idx, us, them)
        for c in range(NCORES)
    ]
    res = run_bass_kernel_spmd(nc, in_maps, list(range(NCORES)), trace=trace)
    outp = np.concatenate([res.results[c]["out"] for c in range(NCORES)], axis=0)
    return outp.astype(np.float32), res


def _host_fallback(w_indices, w_values, b_indices, b_values, us, them,
                   ft_max_val, merged_weight, bias):
    def acc(idx, val):
        rows = merged_weight[idx]
        return np.einsum('bk,bkd->bd', val, rows) + bias
    w = acc(w_indices, w_values)
    b = acc(b_indices, b_values)
    wacc, wpsqt = w[:, :L1], w[:, L1:]
    bacc, bpsqt = b[:, :L1], b[:, L1:]
    l0 = us * np.concatenate([wacc, bacc], axis=1) \
        + them * np.concatenate([bacc, wacc], axis=1)
    l0 = np.clip(l0, 0.0, np.float32(float(np.asarray(ft_max_val))))
    psqt = (wpsqt - bpsqt) * (us - 0.5)
    return np.concatenate([l0, psqt], axis=1).astype(np.float32)


def kernel(w_indices, w_values, b_indices, b_values, us, them, ft_max_val,
           merged_weight, bias):
    if not (np.all(np.asarray(w_values) == 1.0)
            and np.all(np.asarray(b_values) == 1.0)):
        # the device program folds the unit feature values into plain
        # accumulation; anything else is out of spec — stay correct on host
        return _host_fallback(w_indices, w_values, b_indices, b_values, us,
                              them, ft_max_val, merged_weight, bias)
    outp, _ = run_on_hw(w_indices, w_values, b_indices, b_values, us, them,
                        ft_max_val, merged_weight, bias)
    return outp


# revision 6
# speedup vs baseline: 1.0142x; 1.0142x over previous
"""Trainium2 Bass kernel for nn_ComposedFeatureTransformer (NNUE-style double
feature transformer: sparse gather-accumulate + bias, perspective concat, clip,
psqt head).

Strategy: data-parallel over batch across 8 NeuronCores (512 items/core, table
replicated). Per core, items are processed in 4 tiles of 128 (one item per SBUF
partition); each tile needs two 32-row weighted sums (w/b perspectives).

The table is cast to fp16 on host: the kernel is memory-bound on gather
traffic, and halving bytes-per-row halves DMA time; fp16 accumulation keeps
absmax error ~2.4e-3 of output scale (tolerance 2e-2). Rows are fetched with
indirect DMA (one [128, D] fp16 row-gather per feature slot) into a ring of
staging tiles; DVE accumulates staging tiles into the per-perspective
accumulator (first add seeds with the broadcast bias; fp16 operands get the
DVE 2x packed mode), then combines perspectives (us/them mix + clip + psqt)
into an fp16 output tile that SP (HWDGE) streams to DRAM; host upcasts to f32.
All DMA writes are plain copies — no in-DMA accumulate (the SDMA CCE
read-modify-write is unsound for multi-packet rows). Per-core gather traffic:
512*64 rows * 6160 B ~= 202 MB.

Requires w_values/b_values == 1 (guaranteed by the problem spec fill); falls
back to a host computation otherwise so kernel() stays correct on any input.
"""
import sys

if '/opt/trn_rl_repo' not in sys.path:
    sys.path.insert(0, '/opt/trn_rl_repo')

import numpy as np

import concourse.bass as bass
import concourse.mybir as mybir
from concourse.bass_utils import run_bass_kernel_spmd

L1 = 3072
NPSQT = 8
D = L1 + NPSQT            # 3080
V = 45056                 # table rows
K = 32                    # active features per perspective
B = 4096                  # batch
NCORES = 8
BPC = B // NCORES         # 512 items per core
NT = BPC // 128           # 4 item-tiles per core
OUTD = 2 * L1 + NPSQT     # 6152
# PAIR=2 (two rows per indirect DMA) was measured cost-model-neutral —
# descriptor generation is fully hidden behind the transfer stream — so the
# hardware-validated single-row configuration is kept.
PAIR = 1                  # table rows fetched per indirect DMA
S = 6                     # staging ring depth (slots of PAIR rows)

f32 = mybir.dt.float32
f16 = mybir.dt.float16
i32 = mybir.dt.int32

_CACHE = {}


def build_nc(ft_max: float, repeat: int = 1):
    """Build the single-core Bass program (shared SPMD across all 8 cores).

    repeat>1 re-runs the whole compute that many times (for timing the
    on-device portion by slope; the output is simply rewritten).
    """
    nc = bass.Bass()
    table = nc.declare_dram_parameter("table", [V, D], f16, isOutput=False)
    biasb = nc.declare_dram_parameter("biasb", [128, D], f16, isOutput=False)
    idxs = nc.declare_dram_parameter("idxs", [128, 2 * NT * K], i32, isOutput=False)
    usth = nc.declare_dram_parameter("usth", [128, 3 * NT], f32, isOutput=False)
    out = nc.declare_dram_parameter("out", [BPC, OUTD], f16, isOutput=True)

    NG = repeat * 2 * NT * K      # total gathers
    LOADS = 48                    # idx + usth + bias preload sem ticks

    from contextlib import ExitStack
    with ExitStack() as ctx:
        idx_s = ctx.enter_context(nc.sbuf_tensor([128, 2 * NT * K], i32))
        usth_s = ctx.enter_context(nc.sbuf_tensor([128, 3 * NT], f32))
        bias_s = ctx.enter_context(nc.sbuf_tensor([128, D], f16))
        acc_w = ctx.enter_context(nc.sbuf_tensor([128, D], f16))
        acc_b = ctx.enter_context(nc.sbuf_tensor([128, D], f16))
        stage = ctx.enter_context(nc.sbuf_tensor([128, S * PAIR * D], f16))
        out_t0 = ctx.enter_context(nc.sbuf_tensor([128, OUTD], f16))
        out_t1 = ctx.enter_context(nc.sbuf_tensor([128, OUTD], f16))
        tmp = ctx.enter_context(nc.sbuf_tensor([128, L1], f16))
        tmp2 = ctx.enter_context(nc.sbuf_tensor([128, L1], f16))
        load_sem = ctx.enter_context(nc.semaphore("load_sem"))
        # one completion sem per staging slot / output-tile parity: a waiter's
        # threshold can only be satisfied by that slot's own DMA chain, so a
        # fast SDMA engine running ahead on *other* DMAs can't mask a slow
        # engine's unfinished descriptors (cumulative counts on one shared sem
        # are unsound across concurrently in-flight DMAs)
        gsem = [ctx.enter_context(nc.semaphore(f"gsem{i}")) for i in range(S)]
        dve_sem = ctx.enter_context(nc.semaphore("dve_sem"))
        combine_sem = ctx.enter_context(nc.semaphore("combine_sem"))
        osem0 = ctx.enter_context(nc.semaphore("osem0"))
        osem1 = ctx.enter_context(nc.semaphore("osem1"))
        block = ctx.enter_context(nc.Block())
        osem = [osem0, osem1]
        out_t = [out_t0, out_t1]

        def stg(s, j=0):
            # row j of staging slot s
            return stage[:, (s * PAIR + j) * D:(s * PAIR + j + 1) * D]

        @block.gpsimd
        def _(g):
            g.dma_start(out=idx_s[:], in_=idxs[:]).then_inc(load_sem, 16)
            g.dma_start(out=usth_s[:], in_=usth[:]).then_inc(load_sem, 16)
            g.dma_start(out=bias_s[:], in_=biasb[:]).then_inc(load_sem, 16)
            # descriptor generation reads idx_s from SBUF: loads must land first
            g.wait_ge(load_sem, LOADS)
            assert K % PAIR == 0  # a DMA's rows never straddle units
            for gp in range(NG // PAIR):
                u, k0 = (gp * PAIR) // K, (gp * PAIR) % K
                t, p = (u // 2) % NT, u % 2
                col = (2 * t + p) * K + k0
                if gp >= S:
                    # DVE must have consumed the staging slot's previous tenant
                    g.wait_ge(dve_sem, (gp - S + 1) * PAIR)
                g.indirect_dma_start(
                    out=stage[:, (gp % S) * PAIR * D:((gp % S) + 1) * PAIR * D],
                    out_offset=None,
                    in_=table[:],
                    in_offset=bass.IndirectOffsetOnAxis(
                        ap=idx_s[:, col:col + PAIR], axis=0
                    ),
                ).then_inc(gsem[gp % S], 16)

        @block.vector
        def _(v):
            v.wait_ge(load_sem, LOADS)   # usth_s/bias_s resident
            for it in range(repeat * NT):
                t, pb = it % NT, it % 2
                w, b, o = acc_w, acc_b, out_t[pb]
                us = usth_s[:, t:t + 1]
                them = usth_s[:, NT + t:NT + t + 1]
                ush = usth_s[:, 2 * NT + t:2 * NT + t + 1]
                # b-perspective rows are gathered first so them*b / us*b can
                # be precomputed while the w rows are still streaming in —
                # that keeps only ~4 DVE ops after the final w add (tail)
                for p, acc in ((0, acc_b), (1, acc_w)):
                    u = 2 * it + p
                    for k in range(K):
                        gi = u * K + k
                        gp, j = gi // PAIR, gi % PAIR
                        if j == 0:
                            v.wait_ge(gsem[gp % S], 16 * (gp // S + 1))
                        src = bias_s if k == 0 else acc
                        v.tensor_tensor(
                            out=acc[:], in0=src[:], in1=stg(gp % S, j),
                            op=mybir.AluOpType.add,
                        ).then_inc(dve_sem, 1)
                    if p == 0:
                        v.tensor_scalar_mul(tmp[:], b[:, :L1], them)
                        v.tensor_scalar_mul(tmp2[:], b[:, :L1], us)
                if it >= 2:
                    # SP must have drained out_t[pb] (tile it-2, both halves)
                    v.wait_ge(osem[pb], 32 * ((it - 2) // 2 + 1))
                # o[:, :L1] = clip(us*w + them*b); o[:, L1:2L1] = clip(us*b + them*w)
                v.tensor_scalar_mul(o[:, 0:L1], w[:, :L1], us)
                v.tensor_tensor(
                    out=o[:, 0:L1], in0=o[:, 0:L1], in1=tmp[:],
                    op=mybir.AluOpType.add,
                )
                v.tensor_scalar(
                    o[:, 0:L1], o[:, 0:L1], 0.0, ft_max,
                    op0=mybir.AluOpType.max, op1=mybir.AluOpType.min,
                ).then_inc(combine_sem, 1)      # half 1 (cols [0, L1)) ready
                v.tensor_scalar_mul(o[:, L1:2 * L1], w[:, :L1], them)
                v.tensor_tensor(
                    out=o[:, L1:2 * L1], in0=o[:, L1:2 * L1], in1=tmp2[:],
                    op=mybir.AluOpType.add,
                )
                v.tensor_scalar(
                    o[:, L1:2 * L1], o[:, L1:2 * L1], 0.0, ft_max,
                    op0=mybir.AluOpType.max, op1=mybir.AluOpType.min,
                )
                # psqt = (w_psqt - b_psqt) * (us - 0.5); bias cancels in the diff
                v.tensor_tensor(
                    out=tmp[:, :NPSQT], in0=w[:, L1:D], in1=b[:, L1:D],
                    op=mybir.AluOpType.subtract,
                )
                v.tensor_scalar_mul(
                    o[:, 2 * L1:OUTD], tmp[:, :NPSQT], ush
                ).then_inc(combine_sem, 1)      # half 2 (cols [L1, OUTD)) ready

        @block.sync
        def _(s):
            for it in range(repeat * NT):
                t, pb = it % NT, it % 2
                # two half-row DMAs per tile: the first half streams out while
                # DVE is still combining the second half
                s.wait_ge(combine_sem, 2 * it + 1)
                s.dma_start(
                    out=out[t * 128:(t + 1) * 128, 0:L1],
                    in_=out_t[pb][:, 0:L1],
                ).then_inc(osem[pb], 16)
                s.wait_ge(combine_sem, 2 * it + 2)
                s.dma_start(
                    out=out[t * 128:(t + 1) * 128, L1:OUTD],
                    in_=out_t[pb][:, L1:OUTD],
                ).then_inc(osem[pb], 16)
            n = repeat * NT
            s.wait_ge(osem[0], 32 * ((n + 1) // 2))
            s.wait_ge(osem[1], 32 * (n // 2))

    return nc


def _prep_core_inputs(c, table, biasb, w_idx, b_idx, us, them):
    sl = slice(c * BPC, (c + 1) * BPC)
    wi = w_idx[sl].reshape(NT, 128, K)
    bi = b_idx[sl].reshape(NT, 128, K)
    blocks = []
    for t in range(NT):
        # b-perspective first: matches the device accumulate order (u even =
        # b rows, u odd = w rows) so them*b/us*b precompute overlaps w gathers
        blocks.append(bi[t])
        blocks.append(wi[t])
    idxs = np.ascontiguousarray(np.concatenate(blocks, axis=1), dtype=np.int32)
    us_c = np.ascontiguousarray(us[sl, 0].reshape(NT, 128).T, dtype=np.float32)
    th_c = np.ascontiguousarray(them[sl, 0].reshape(NT, 128).T, dtype=np.float32)
    usth = np.concatenate([us_c, th_c, us_c - 0.5], axis=1).astype(np.float32)
    return {"table": table, "biasb": biasb, "idxs": idxs, "usth": usth}


def run_on_hw(w_indices, w_values, b_indices, b_values, us, them, ft_max_val,
              merged_weight, bias, trace=False, repeat=1):
    """Run the device kernel; returns (output [B, OUTD], BassKernelResults)."""
    ft_max = float(np.asarray(ft_max_val))
    key = ("nc", ft_max, repeat)
    if key not in _CACHE:
        _CACHE[key] = build_nc(ft_max, repeat)
    nc = _CACHE[key]

    table = np.ascontiguousarray(merged_weight, dtype=np.float16)
    biasb = np.ascontiguousarray(
        np.broadcast_to(np.asarray(bias, dtype=np.float16), (128, D))
    )
    w_idx = np.asarray(w_indices, dtype=np.int64)
    b_idx = np.asarray(b_indices, dtype=np.int64)
    us = np.asarray(us, dtype=np.float32)
    them = np.asarray(them, dtype=np.float32)

    in_maps = [
        _prep_core_inputs(c, table, biasb, w_idx, b_idx, us, them)
        for c in range(NCORES)
    ]
    res = run_bass_kernel_spmd(nc, in_maps, list(range(NCORES)), trace=trace)
    outp = np.concatenate([res.results[c]["out"] for c in range(NCORES)], axis=0)
    return outp.astype(np.float32), res


def _host_fallback(w_indices, w_values, b_indices, b_values, us, them,
                   ft_max_val, merged_weight, bias):
    def acc(idx, val):
        rows = merged_weight[idx]
        return np.einsum('bk,bkd->bd', val, rows) + bias
    w = acc(w_indices, w_values)
    b = acc(b_indices, b_values)
    wacc, wpsqt = w[:, :L1], w[:, L1:]
    bacc, bpsqt = b[:, :L1], b[:, L1:]
    l0 = us * np.concatenate([wacc, bacc], axis=1) \
        + them * np.concatenate([bacc, wacc], axis=1)
    l0 = np.clip(l0, 0.0, np.float32(float(np.asarray(ft_max_val))))
    psqt = (wpsqt - bpsqt) * (us - 0.5)
    return np.concatenate([l0, psqt], axis=1).astype(np.float32)


def kernel(w_indices, w_values, b_indices, b_values, us, them, ft_max_val,
           merged_weight, bias):
    if not (np.all(np.asarray(w_values) == 1.0)
            and np.all(np.asarray(b_values) == 1.0)):
        # the device program folds the unit feature values into plain
        # accumulation; anything else is out of spec — stay correct on host
        return _host_fallback(w_indices, w_values, b_indices, b_values, us,
                              them, ft_max_val, merged_weight, bias)
    outp, _ = run_on_hw(w_indices, w_values, b_indices, b_values, us, them,
                        ft_max_val, merged_weight, bias)
    return outp
